# revision 10
# baseline (speedup 1.0000x reference)
"""Trainium2 Bass kernel for nn_ComplexHoloLinear.

Computes out = x @ Wr.T + cos(phase)[batch] * (x @ Wi.T) where Wr/Wi are
dense [4096, 4096] matrices assembled on-device by summing COO duplicate
"generation" layers (scatter-add via CCE accumulate-DMA).

Distribution: output-feature sharding. Each of the 8 cores owns 512 output
rows; it assembles its W.T slice in SBUF, computes cos(phase) on-device
(DVE range-fold + ACT Sin LUT), then for each of the 4 batches builds the
combined weight W_b = Wr + cos_b * Wi in SBUF and streams all 8192 tokens
of xT through the PE (PSUM-accumulated over the 32 feature chunks).

Host side does layout only: transposes x, sorts the COO edges by cell, and
places the values into per-generation dense layers in the exact SBUF layout
(plus folding the tiny >=2nd-duplicate tail, 0.2% of edges).

Two precisions: "fp16" (default — half the HBM traffic, 2x PE rate,
~7e-4 rel err) and "f32r" (TF32-style full-rate fp32, ~2e-4 rel err).
"""

import math
from contextlib import ExitStack

import numpy as np

import concourse.bass as bass
import concourse.tile as tile
from concourse import bacc, mybir

F32 = mybir.dt.float32
F32R = mybir.dt.float32r
F16 = mybir.dt.float16
ADD = mybir.AluOpType.add


class Cfg:
    """Full-size problem config. A scaled-down variant is used by tests."""

    NCORES = 8
    NTOK = 8192       # B * S tokens
    NBATCH = 4        # batches (distinct cos factors)
    F = 4096          # in features (contraction)
    RTOT = 4096       # out features
    TOKG = 512        # tokens per matmul sweep group (4 psum tiles of 128)
    ASM_GRP = 2       # W chunks per assembly DMA group
    PREC = "fp16"     # "fp16" | "f32r"

    @property
    def RSH(self):    # rows per core
        return self.RTOT // self.NCORES

    @property
    def NK(self):     # feature chunks of 128
        return self.F // 128

    @property
    def NTG(self):    # token groups
        return self.NTOK // self.TOKG

    @property
    def WFREE(self):  # W tile free size
        return self.NK * self.RSH

    @property
    def DT_NP(self):
        return np.float16 if self.PREC == "fp16" else np.float32

    @property
    def DT(self):
        return F16 if self.PREC == "fp16" else F32


def build_body(ctx: ExitStack, tc: tile.TileContext, cfg: Cfg, aps: dict):
    nc = tc.nc
    xT = aps["xT"]          # [F, NTOK]
    l0r, l1r, l2r = aps["l0r"], aps["l1r"], aps["l2r"]  # [128, WFREE]
    l0i, l1i, l2i = aps["l0i"], aps["l1i"], aps["l2i"]
    phase = aps["phase"]    # [1, NBATCH]
    out = aps["out"]        # [NTOK, RSH]

    RSH, NK, NB = cfg.RSH, cfg.NK, cfg.NBATCH
    TPG = cfg.TOKG // 128   # psum tiles per token group
    fp16 = cfg.PREC == "fp16"
    DT = cfg.DT

    wpool = ctx.enter_context(tc.tile_pool(name="w", bufs=1))
    xpool = ctx.enter_context(tc.tile_pool(name="x", bufs=12))
    tpool = ctx.enter_context(tc.tile_pool(name="tmp", bufs=3))
    spool = ctx.enter_context(tc.tile_pool(name="stage", bufs=3))
    mpool = ctx.enter_context(tc.tile_pool(name="misc", bufs=1))
    pspool = ctx.enter_context(tc.tile_pool(name="ps", bufs=2, space="PSUM"))
    if not fp16:
        xrpool = ctx.enter_context(tc.tile_pool(name="xr", bufs=4))

    # --- cos(phase) on device: fold phase+pi/2 into [-pi, pi], then Sin LUT
    ph = mpool.tile([128, NB], F32)
    nc.sync.dma_start(out=ph[:], in_=phase[:1, :].to_broadcast([128, NB]))
    q = mpool.tile([128, NB], F32)
    nc.vector.tensor_scalar_add(q[:], ph[:], math.pi / 2)
    msk = mpool.tile([128, NB], F32)
    nc.vector.tensor_scalar(
        out=msk[:], in0=q[:], scalar1=math.pi, scalar2=2 * math.pi,
        op0=mybir.AluOpType.is_gt, op1=mybir.AluOpType.mult,
    )
    nc.vector.tensor_tensor(out=q[:], in0=q[:], in1=msk[:],
                            op=mybir.AluOpType.subtract)
    cos_t = mpool.tile([128, NB], F32)
    nc.scalar.activation(cos_t[:], q[:], mybir.ActivationFunctionType.Sin)

    if fp16:
        # --- assemble Wr and Wi slices in SBUF (fp16). Assembly copies ride
        # the scalar HWDGE ring so the sync ring is free for xt loads.
        WR = wpool.tile([128, cfg.WFREE], DT)
        WI = wpool.tile([128, cfg.WFREE], DT)
        gw = cfg.ASM_GRP * RSH
        for g in range(NK // cfg.ASM_GRP):
            sl = slice(g * gw, (g + 1) * gw)
            nc.scalar.dma_start(out=WR[:, sl], in_=l0r[:, sl])
            nc.gpsimd.dma_start(out=WR[:, sl], in_=l1r[:, sl], accum_op=ADD)
            nc.gpsimd.dma_start(out=WR[:, sl], in_=l2r[:, sl], accum_op=ADD)
            nc.scalar.dma_start(out=WI[:, sl], in_=l0i[:, sl])
            nc.gpsimd.dma_start(out=WI[:, sl], in_=l1i[:, sl], accum_op=ADD)
            nc.gpsimd.dma_start(out=WI[:, sl], in_=l2i[:, sl], accum_op=ADD)
        WB = wpool.tile([128, cfg.WFREE], DT)
    else:
        # f32r: WB doubles as Wr accumulator, combined incrementally.
        # DMA-written (unrounded) WB is only consumed by the DVE combine,
        # which rewrites it f32r-rounded before any matmul reads it.
        WB = wpool.tile([128, cfg.WFREE], F32R)
        WI = wpool.tile([128, cfg.WFREE], F32)
        gw = cfg.ASM_GRP * RSH
        for g in range(NK // cfg.ASM_GRP):
            sl = slice(g * gw, (g + 1) * gw)
            nc.scalar.dma_start(out=WB[:, sl], in_=l0r[:, sl].bitcast(F32R))
            nc.gpsimd.dma_start(out=WB[:, sl], in_=l1r[:, sl].bitcast(F32R),
                                accum_op=ADD)
            nc.gpsimd.dma_start(out=WB[:, sl], in_=l2r[:, sl].bitcast(F32R),
                                accum_op=ADD)
            nc.scalar.dma_start(out=WI[:, sl], in_=l0i[:, sl])
            nc.gpsimd.dma_start(out=WI[:, sl], in_=l1i[:, sl], accum_op=ADD)
            nc.gpsimd.dma_start(out=WI[:, sl], in_=l2i[:, sl], accum_op=ADD)

    # delta[b]: fp16 path uses absolute combine (WB = WR + cos_b*WI);
    # f32r path uses incremental (WB += (cos_b - cos_{b-1})*WI).
    dlt = mpool.tile([128, NB], F32)
    if fp16:
        nc.vector.tensor_copy(dlt[:], cos_t[:])
    else:
        nc.vector.tensor_copy(dlt[:, 0:1], cos_t[:, 0:1])
        if NB > 1:
            nc.vector.tensor_tensor(out=dlt[:, 1:NB], in0=cos_t[:, 1:NB],
                                    in1=cos_t[:, 0:NB - 1],
                                    op=mybir.AluOpType.subtract)

    # --- per batch: build W_b, then matmul all tokens of the batch
    ntg_per_b = cfg.NTG // NB
    for b in range(NB):
        for k in range(NK):
            sl = slice(k * RSH, (k + 1) * RSH)
            tmp = tpool.tile([128, RSH], DT)
            nc.vector.tensor_scalar(out=tmp[:], in0=WI[:, sl],
                                    scalar1=dlt[:, b:b + 1], scalar2=None,
                                    op0=mybir.AluOpType.mult)
            if fp16:
                nc.vector.tensor_tensor(out=WB[:, sl], in0=WR[:, sl],
                                        in1=tmp[:], op=ADD)
            else:
                nc.vector.tensor_tensor(out=WB[:, sl],
                                        in0=WB[:, sl].bitcast(F32),
                                        in1=tmp[:], op=ADD)
        for tg in range(ntg_per_b):
            gt = b * ntg_per_b + tg
            pts = [pspool.tile([128, RSH], F32, space="PSUM", tag=f"ps{t}",
                               name=f"ps{t}")
                   for t in range(TPG)]
            for k in range(NK):
                xt = xpool.tile([128, cfg.TOKG], DT)
                dma_eng = nc.sync if (k % 2 == 0) else nc.scalar
                row0 = (k * cfg.NTG + gt) * 128
                dma_eng.dma_start(out=xt[:], in_=xT[row0:row0 + 128, :])
                if fp16:
                    lhs_tile = xt
                else:
                    lhs_tile = xrpool.tile([128, cfg.TOKG], F32R, name="xtr")
                    nc.scalar.activation(lhs_tile[:], xt[:],
                                         mybir.ActivationFunctionType.Copy)
                for t in range(TPG):
                    nc.tensor.matmul(
                        out=pts[t][:],
                        lhsT=lhs_tile[:, t * 128:(t + 1) * 128],
                        rhs=WB[:, k * RSH:(k + 1) * RSH],
                        start=(k == 0), stop=(k == NK - 1),
                    )
            for t in range(TPG):
                stg = spool.tile([128, RSH], F32)
                nc.scalar.copy(out=stg[:], in_=pts[t][:])
                tok0 = gt * cfg.TOKG + t * 128
                nc.gpsimd.dma_start(out=out[tok0:tok0 + 128, :], in_=stg[:])


def build_nc(cfg: Cfg):
    nc = bacc.Bacc("TRN2", target_bir_lowering=False, debug=False,
                   num_devices=cfg.NCORES)
    aps = {
        # xT pre-tiled on host: row block (k*NTG + gt)*128 holds the
        # [128 feat, TOKG tok] tile for feature-chunk k, token-group gt.
        "xT": nc.dram_tensor("xT", [cfg.NK * cfg.NTG * 128, cfg.TOKG], cfg.DT,
                             kind="ExternalInput").ap(),
        "phase": nc.dram_tensor("phase", [1, cfg.NBATCH], F32,
                                kind="ExternalInput").ap(),
        "out": nc.dram_tensor("out", [cfg.NTOK, cfg.RSH], F32,
                              kind="ExternalOutput").ap(),
    }
    for name in ("l0r", "l1r", "l2r", "l0i", "l1i", "l2i"):
        aps[name] = nc.dram_tensor(name, [128, cfg.WFREE], cfg.DT,
                                   kind="ExternalInput").ap()
    with tile.TileContext(nc) as tc:
        with ExitStack() as ctx:
            build_body(ctx, tc, cfg, aps)
    nc.compile()
    return nc


def host_prep(cfg: Cfg, x, rows, cols, w_real, w_imag, phase_angles):
    """Pure-layout host prep: transpose x; sort COO edges by cell and place
    values into 3 per-generation dense layers in the on-chip W.T layout.
    Returns per-core input maps."""
    x = np.ascontiguousarray(np.asarray(x, dtype=np.float32)).reshape(
        cfg.NTOK, cfg.F)
    xT = x.T.astype(cfg.DT_NP)  # [F, NTOK]
    # pre-tile: row block (k*NTG + gt)*128 = [128 feat, TOKG tok] tile
    xT = np.ascontiguousarray(
        xT.reshape(cfg.NK, 128, cfg.NTG, cfg.TOKG).transpose(0, 2, 1, 3)
    ).reshape(cfg.NK * cfg.NTG * 128, cfg.TOKG)

    rows = np.asarray(rows).astype(np.int32, copy=False)
    cols = np.asarray(cols).astype(np.int32, copy=False)
    w_real = np.asarray(w_real, dtype=cfg.DT_NP)
    w_imag = np.asarray(w_imag, dtype=cfg.DT_NP)

    colbits = int(np.log2(cfg.F))
    lin = (rows.astype(np.int64) << colbits) | cols
    if cfg.RTOT * cfg.F <= 2**31:
        lin = lin.astype(np.int32)
    order = np.argsort(lin, kind="stable")
    sl = lin[order]
    wr_s = w_real[order]
    wi_s = w_imag[order]

    n = len(sl)
    starts = np.empty(n, dtype=bool)
    starts[0] = True
    starts[1:] = sl[1:] != sl[:-1]
    idx = np.arange(n, dtype=np.int64)
    gen = idx - np.maximum.accumulate(np.where(starts, idx, 0))

    r = (sl.astype(np.int64) >> colbits)
    c = (sl.astype(np.int64) & (cfg.F - 1))
    rsh_bits = int(np.log2(cfg.RSH))
    core = r >> rsh_bits
    p = c & 127
    off = ((c >> 7) << rsh_bits) + (r & (cfg.RSH - 1))

    shp = (cfg.NCORES, 128, cfg.WFREE)
    layers = {name: np.zeros(shp, dtype=cfg.DT_NP)
              for name in ("l0r", "l1r", "l2r", "l0i", "l1i", "l2i")}
    for g, (nr, ni) in enumerate((("l0r", "l0i"), ("l1r", "l1i"))):
        m = gen == g
        layers[nr][core[m], p[m], off[m]] = wr_s[m]
        layers[ni][core[m], p[m], off[m]] = wi_s[m]
    m = gen >= 2
    np.add.at(layers["l2r"], (core[m], p[m], off[m]), wr_s[m])
    np.add.at(layers["l2i"], (core[m], p[m], off[m]), wi_s[m])

    phase_in = np.asarray(phase_angles, dtype=np.float32).reshape(1, cfg.NBATCH)

    in_maps = []
    for cid in range(cfg.NCORES):
        m = {"xT": xT, "phase": phase_in}
        for name, arr in layers.items():
            m[name] = arr[cid]
        in_maps.append(m)
    return in_maps


_NC_CACHE = {}
LAST_RESULTS = None  # BassKernelResults of the most recent kernel() call


def kernel(x, rows, cols, w_real, w_imag, phase_angles, out_features=4096,
           **_ignored):
    from concourse.bass_utils import run_bass_kernel_spmd

    global LAST_RESULTS
    cfg = Cfg()
    assert int(out_features) == cfg.RTOT

    if "nc" not in _NC_CACHE:
        _NC_CACHE["nc"] = build_nc(cfg)
    nc = _NC_CACHE["nc"]

    in_maps = host_prep(cfg, x, rows, cols, w_real, w_imag, phase_angles)
    res = run_bass_kernel_spmd(nc, in_maps, core_ids=list(range(cfg.NCORES)))
    LAST_RESULTS = res
    out = np.concatenate([res.results[c]["out"] for c in range(cfg.NCORES)],
                         axis=1)
    return out.reshape(cfg.NTOK // 2048, 2048, cfg.RTOT)


# revision 13
# speedup vs baseline: 1.2303x; 1.2303x over previous
"""Trainium2 Bass kernel for nn_ComplexHoloLinear.

Computes out = x @ Wr.T + cos(phase)[batch] * (x @ Wi.T) where Wr/Wi are
dense [4096, 4096] matrices assembled on-device by summing COO duplicate
"generation" layers (scatter-add via CCE accumulate-DMA).

Distribution: output-feature sharding. Each of the 8 cores owns 512 output
rows; it assembles its W.T slice in SBUF, computes cos(phase) on-device
(DVE range-fold + ACT Sin LUT), then for each of the 4 batches builds the
combined weight W_b = Wr + cos_b * Wi in SBUF and streams all 8192 tokens
of xT through the PE (PSUM-accumulated over the 32 feature chunks).

Host side does layout only: transposes x, sorts the COO edges by cell, and
places the values into per-generation dense layers in the exact SBUF layout
(plus folding the tiny >=2nd-duplicate tail, 0.2% of edges).

Two precisions: "fp16" (default — half the HBM traffic, 2x PE rate,
~7e-4 rel err) and "f32r" (TF32-style full-rate fp32, ~2e-4 rel err).
"""

import math
from contextlib import ExitStack

import numpy as np

import concourse.bass as bass
import concourse.tile as tile
from concourse import bacc, mybir

F32 = mybir.dt.float32
F32R = mybir.dt.float32r
F16 = mybir.dt.float16
ADD = mybir.AluOpType.add


class Cfg:
    """Full-size problem config. A scaled-down variant is used by tests."""

    NCORES = 8
    NTOK = 8192       # B * S tokens
    NBATCH = 4        # batches (distinct cos factors)
    F = 4096          # in features (contraction)
    RTOT = 4096       # out features
    TOKG = 512        # tokens per matmul sweep group (4 psum tiles of 128)
    ASM_GRP = 4       # W chunks per assembly DMA group
    PREC = "fp16"     # "fp16" | "f32r"
    ASM_MODE = "dve"  # "dve" (plain DMA + DVE adds) | "cce" (accumulate-DMA)

    @property
    def RSH(self):    # rows per core
        return self.RTOT // self.NCORES

    @property
    def NK(self):     # feature chunks of 128
        return self.F // 128

    @property
    def NTG(self):    # token groups
        return self.NTOK // self.TOKG

    @property
    def WFREE(self):  # W tile free size
        return self.NK * self.RSH

    @property
    def DT_NP(self):
        return np.float16 if self.PREC == "fp16" else np.float32

    @property
    def DT(self):
        return F16 if self.PREC == "fp16" else F32


def build_body(ctx: ExitStack, tc: tile.TileContext, cfg: Cfg, aps: dict):
    nc = tc.nc
    xT = aps["xT"]          # [F, NTOK]
    l0r, l1r, l2r = aps["l0r"], aps["l1r"], aps["l2r"]  # [128, WFREE]
    l0i, l1i, l2i = aps["l0i"], aps["l1i"], aps["l2i"]
    phase = aps["phase"]    # [1, NBATCH]
    out = aps["out"]        # [NTOK, RSH]

    RSH, NK, NB = cfg.RSH, cfg.NK, cfg.NBATCH
    TPG = cfg.TOKG // 128   # psum tiles per token group
    fp16 = cfg.PREC == "fp16"
    DT = cfg.DT

    wpool = ctx.enter_context(tc.tile_pool(name="w", bufs=1))
    xpool = ctx.enter_context(tc.tile_pool(name="x", bufs=8))
    tpool = ctx.enter_context(tc.tile_pool(name="tmp", bufs=3))
    spool = ctx.enter_context(tc.tile_pool(name="stage", bufs=3))
    mpool = ctx.enter_context(tc.tile_pool(name="misc", bufs=1))
    pspool = ctx.enter_context(tc.tile_pool(name="ps", bufs=2, space="PSUM"))
    if not fp16:
        xrpool = ctx.enter_context(tc.tile_pool(name="xr", bufs=4))

    # --- cos(phase) on device: fold phase+pi/2 into [-pi, pi], then Sin LUT
    ph = mpool.tile([128, NB], F32)
    nc.sync.dma_start(out=ph[:], in_=phase[:1, :].to_broadcast([128, NB]))
    q = mpool.tile([128, NB], F32)
    nc.vector.tensor_scalar_add(q[:], ph[:], math.pi / 2)
    msk = mpool.tile([128, NB], F32)
    nc.vector.tensor_scalar(
        out=msk[:], in0=q[:], scalar1=math.pi, scalar2=2 * math.pi,
        op0=mybir.AluOpType.is_gt, op1=mybir.AluOpType.mult,
    )
    nc.vector.tensor_tensor(out=q[:], in0=q[:], in1=msk[:],
                            op=mybir.AluOpType.subtract)
    cos_t = mpool.tile([128, NB], F32)
    nc.scalar.activation(cos_t[:], q[:], mybir.ActivationFunctionType.Sin)

    if fp16:
        # --- assemble Wr and Wi slices in SBUF (fp16). Assembly copies ride
        # the scalar HWDGE ring so the sync ring is free for xt loads.
        WR = wpool.tile([128, cfg.WFREE], DT)
        WI = wpool.tile([128, cfg.WFREE], DT)
        gw = cfg.ASM_GRP * RSH
        if cfg.ASM_MODE == "dve":
            # plain full-rate DMAs; duplicate-layer summation on DVE
            lpool = ctx.enter_context(tc.tile_pool(name="lscr", bufs=4))
            for g in range(NK // cfg.ASM_GRP):
                sl = slice(g * gw, (g + 1) * gw)
                for W, ls in ((WR, (l0r, l1r, l2r)), (WI, (l0i, l1i, l2i))):
                    nc.scalar.dma_start(out=W[:, sl], in_=ls[0][:, sl])
                    for l_ap in ls[1:]:
                        scr = lpool.tile([128, gw], DT, name="scr")
                        nc.gpsimd.dma_start(out=scr[:], in_=l_ap[:, sl])
                        nc.vector.tensor_tensor(out=W[:, sl], in0=W[:, sl],
                                                in1=scr[:], op=ADD)
        else:
            for g in range(NK // cfg.ASM_GRP):
                sl = slice(g * gw, (g + 1) * gw)
                nc.scalar.dma_start(out=WR[:, sl], in_=l0r[:, sl])
                nc.gpsimd.dma_start(out=WR[:, sl], in_=l1r[:, sl],
                                    accum_op=ADD)
                nc.gpsimd.dma_start(out=WR[:, sl], in_=l2r[:, sl],
                                    accum_op=ADD)
                nc.scalar.dma_start(out=WI[:, sl], in_=l0i[:, sl])
                nc.gpsimd.dma_start(out=WI[:, sl], in_=l1i[:, sl],
                                    accum_op=ADD)
                nc.gpsimd.dma_start(out=WI[:, sl], in_=l2i[:, sl],
                                    accum_op=ADD)
        WB = wpool.tile([128, cfg.WFREE], DT)
    else:
        # f32r: WB doubles as Wr accumulator, combined incrementally.
        # DMA-written (unrounded) WB is only consumed by the DVE combine,
        # which rewrites it f32r-rounded before any matmul reads it.
        WB = wpool.tile([128, cfg.WFREE], F32R)
        WI = wpool.tile([128, cfg.WFREE], F32)
        gw = cfg.ASM_GRP * RSH
        for g in range(NK // cfg.ASM_GRP):
            sl = slice(g * gw, (g + 1) * gw)
            nc.scalar.dma_start(out=WB[:, sl], in_=l0r[:, sl].bitcast(F32R))
            nc.gpsimd.dma_start(out=WB[:, sl], in_=l1r[:, sl].bitcast(F32R),
                                accum_op=ADD)
            nc.gpsimd.dma_start(out=WB[:, sl], in_=l2r[:, sl].bitcast(F32R),
                                accum_op=ADD)
            nc.scalar.dma_start(out=WI[:, sl], in_=l0i[:, sl])
            nc.gpsimd.dma_start(out=WI[:, sl], in_=l1i[:, sl], accum_op=ADD)
            nc.gpsimd.dma_start(out=WI[:, sl], in_=l2i[:, sl], accum_op=ADD)

    # delta[b]: fp16 path uses absolute combine (WB = WR + cos_b*WI);
    # f32r path uses incremental (WB += (cos_b - cos_{b-1})*WI).
    dlt = mpool.tile([128, NB], F32)
    if fp16:
        nc.vector.tensor_copy(dlt[:], cos_t[:])
    else:
        nc.vector.tensor_copy(dlt[:, 0:1], cos_t[:, 0:1])
        if NB > 1:
            nc.vector.tensor_tensor(out=dlt[:, 1:NB], in0=cos_t[:, 1:NB],
                                    in1=cos_t[:, 0:NB - 1],
                                    op=mybir.AluOpType.subtract)

    # --- per batch: build W_b, then matmul all tokens of the batch
    ntg_per_b = cfg.NTG // NB
    for b in range(NB):
        for k in range(NK):
            sl = slice(k * RSH, (k + 1) * RSH)
            tmp = tpool.tile([128, RSH], DT)
            nc.vector.tensor_scalar(out=tmp[:], in0=WI[:, sl],
                                    scalar1=dlt[:, b:b + 1], scalar2=None,
                                    op0=mybir.AluOpType.mult)
            if fp16:
                nc.vector.tensor_tensor(out=WB[:, sl], in0=WR[:, sl],
                                        in1=tmp[:], op=ADD)
            else:
                nc.vector.tensor_tensor(out=WB[:, sl],
                                        in0=WB[:, sl].bitcast(F32),
                                        in1=tmp[:], op=ADD)
        for tg in range(ntg_per_b):
            gt = b * ntg_per_b + tg
            pts = [pspool.tile([128, RSH], F32, space="PSUM", tag=f"ps{t}",
                               name=f"ps{t}")
                   for t in range(TPG)]
            for k in range(NK):
                xt = xpool.tile([128, cfg.TOKG], DT)
                dma_eng = nc.sync if (k % 2 == 0) else nc.scalar
                row0 = (k * cfg.NTG + gt) * 128
                dma_eng.dma_start(out=xt[:], in_=xT[row0:row0 + 128, :])
                if fp16:
                    lhs_tile = xt
                else:
                    lhs_tile = xrpool.tile([128, cfg.TOKG], F32R, name="xtr")
                    nc.scalar.activation(lhs_tile[:], xt[:],
                                         mybir.ActivationFunctionType.Copy)
                for t in range(TPG):
                    nc.tensor.matmul(
                        out=pts[t][:],
                        lhsT=lhs_tile[:, t * 128:(t + 1) * 128],
                        rhs=WB[:, k * RSH:(k + 1) * RSH],
                        start=(k == 0), stop=(k == NK - 1),
                    )
            for t in range(TPG):
                stg = spool.tile([128, RSH], F32)
                nc.scalar.copy(out=stg[:], in_=pts[t][:])
                tok0 = gt * cfg.TOKG + t * 128
                nc.gpsimd.dma_start(out=out[tok0:tok0 + 128, :], in_=stg[:])


def build_nc(cfg: Cfg):
    nc = bacc.Bacc("TRN2", target_bir_lowering=False, debug=False,
                   num_devices=cfg.NCORES)
    aps = {
        # xT pre-tiled on host: row block (k*NTG + gt)*128 holds the
        # [128 feat, TOKG tok] tile for feature-chunk k, token-group gt.
        "xT": nc.dram_tensor("xT", [cfg.NK * cfg.NTG * 128, cfg.TOKG], cfg.DT,
                             kind="ExternalInput").ap(),
        "phase": nc.dram_tensor("phase", [1, cfg.NBATCH], F32,
                                kind="ExternalInput").ap(),
        "out": nc.dram_tensor("out", [cfg.NTOK, cfg.RSH], F32,
                              kind="ExternalOutput").ap(),
    }
    for name in ("l0r", "l1r", "l2r", "l0i", "l1i", "l2i"):
        aps[name] = nc.dram_tensor(name, [128, cfg.WFREE], cfg.DT,
                                   kind="ExternalInput").ap()
    with tile.TileContext(nc) as tc:
        with ExitStack() as ctx:
            build_body(ctx, tc, cfg, aps)
    nc.compile()
    return nc


def host_prep(cfg: Cfg, x, rows, cols, w_real, w_imag, phase_angles):
    """Pure-layout host prep: transpose x; sort COO edges by cell and place
    values into 3 per-generation dense layers in the on-chip W.T layout.
    Returns per-core input maps."""
    x = np.ascontiguousarray(np.asarray(x, dtype=np.float32)).reshape(
        cfg.NTOK, cfg.F)
    xT = x.T.astype(cfg.DT_NP)  # [F, NTOK]
    # pre-tile: row block (k*NTG + gt)*128 = [128 feat, TOKG tok] tile
    xT = np.ascontiguousarray(
        xT.reshape(cfg.NK, 128, cfg.NTG, cfg.TOKG).transpose(0, 2, 1, 3)
    ).reshape(cfg.NK * cfg.NTG * 128, cfg.TOKG)

    rows = np.asarray(rows).astype(np.int32, copy=False)
    cols = np.asarray(cols).astype(np.int32, copy=False)
    w_real = np.asarray(w_real, dtype=cfg.DT_NP)
    w_imag = np.asarray(w_imag, dtype=cfg.DT_NP)

    colbits = int(np.log2(cfg.F))
    lin = (rows.astype(np.int64) << colbits) | cols
    if cfg.RTOT * cfg.F <= 2**31:
        lin = lin.astype(np.int32)
    order = np.argsort(lin, kind="stable")
    sl = lin[order]
    wr_s = w_real[order]
    wi_s = w_imag[order]

    n = len(sl)
    starts = np.empty(n, dtype=bool)
    starts[0] = True
    starts[1:] = sl[1:] != sl[:-1]
    idx = np.arange(n, dtype=np.int64)
    gen = idx - np.maximum.accumulate(np.where(starts, idx, 0))

    r = (sl.astype(np.int64) >> colbits)
    c = (sl.astype(np.int64) & (cfg.F - 1))
    rsh_bits = int(np.log2(cfg.RSH))
    core = r >> rsh_bits
    p = c & 127
    off = ((c >> 7) << rsh_bits) + (r & (cfg.RSH - 1))

    shp = (cfg.NCORES, 128, cfg.WFREE)
    layers = {name: np.zeros(shp, dtype=cfg.DT_NP)
              for name in ("l0r", "l1r", "l2r", "l0i", "l1i", "l2i")}
    for g, (nr, ni) in enumerate((("l0r", "l0i"), ("l1r", "l1i"))):
        m = gen == g
        layers[nr][core[m], p[m], off[m]] = wr_s[m]
        layers[ni][core[m], p[m], off[m]] = wi_s[m]
    m = gen >= 2
    np.add.at(layers["l2r"], (core[m], p[m], off[m]), wr_s[m])
    np.add.at(layers["l2i"], (core[m], p[m], off[m]), wi_s[m])

    phase_in = np.asarray(phase_angles, dtype=np.float32).reshape(1, cfg.NBATCH)

    in_maps = []
    for cid in range(cfg.NCORES):
        m = {"xT": xT, "phase": phase_in}
        for name, arr in layers.items():
            m[name] = arr[cid]
        in_maps.append(m)
    return in_maps


_NC_CACHE = {}
LAST_RESULTS = None  # BassKernelResults of the most recent kernel() call


def kernel(x, rows, cols, w_real, w_imag, phase_angles, out_features=4096,
           **_ignored):
    from concourse.bass_utils import run_bass_kernel_spmd

    global LAST_RESULTS
    cfg = Cfg()
    assert int(out_features) == cfg.RTOT

    if "nc" not in _NC_CACHE:
        _NC_CACHE["nc"] = build_nc(cfg)
    nc = _NC_CACHE["nc"]

    in_maps = host_prep(cfg, x, rows, cols, w_real, w_imag, phase_angles)
    res = run_bass_kernel_spmd(nc, in_maps, core_ids=list(range(cfg.NCORES)))
    LAST_RESULTS = res
    out = np.concatenate([res.results[c]["out"] for c in range(cfg.NCORES)],
                         axis=1)
    return out.reshape(cfg.NTOK // 2048, 2048, cfg.RTOT)


# revision 14
# speedup vs baseline: 1.2486x; 1.0149x over previous
"""Trainium2 Bass kernel for nn_ComplexHoloLinear.

Computes out = x @ Wr.T + cos(phase)[batch] * (x @ Wi.T) where Wr/Wi are
dense [4096, 4096] matrices assembled on-device by summing COO duplicate
"generation" layers (scatter-add via CCE accumulate-DMA).

Distribution: output-feature sharding. Each of the 8 cores owns 512 output
rows; it assembles its W.T slice in SBUF, computes cos(phase) on-device
(DVE range-fold + ACT Sin LUT), then for each of the 4 batches builds the
combined weight W_b = Wr + cos_b * Wi in SBUF and streams all 8192 tokens
of xT through the PE (PSUM-accumulated over the 32 feature chunks).

Host side does layout only: transposes x, sorts the COO edges by cell, and
places the values into per-generation dense layers in the exact SBUF layout
(plus folding the tiny >=2nd-duplicate tail, 0.2% of edges).

Two precisions: "fp16" (default — half the HBM traffic, 2x PE rate,
~7e-4 rel err) and "f32r" (TF32-style full-rate fp32, ~2e-4 rel err).
"""

import math
from contextlib import ExitStack

import numpy as np

import concourse.bass as bass
import concourse.tile as tile
from concourse import bacc, mybir

F32 = mybir.dt.float32
F32R = mybir.dt.float32r
F16 = mybir.dt.float16
ADD = mybir.AluOpType.add


class Cfg:
    """Full-size problem config. A scaled-down variant is used by tests."""

    NCORES = 8
    NTOK = 8192       # B * S tokens
    NBATCH = 4        # batches (distinct cos factors)
    F = 4096          # in features (contraction)
    RTOT = 4096       # out features
    TOKG = 512        # tokens per matmul sweep group (4 psum tiles of 128)
    ASM_GRP = 4       # W chunks per assembly DMA group
    PREC = "fp16"     # "fp16" | "f32r"
    ASM_MODE = "dve"  # "dve" (plain DMA + DVE adds) | "cce" (accumulate-DMA)

    @property
    def RSH(self):    # rows per core
        return self.RTOT // self.NCORES

    @property
    def NK(self):     # feature chunks of 128
        return self.F // 128

    @property
    def NTG(self):    # token groups
        return self.NTOK // self.TOKG

    @property
    def WFREE(self):  # W tile free size
        return self.NK * self.RSH

    @property
    def DT_NP(self):
        return np.float16 if self.PREC == "fp16" else np.float32

    @property
    def DT(self):
        return F16 if self.PREC == "fp16" else F32


def build_body(ctx: ExitStack, tc: tile.TileContext, cfg: Cfg, aps: dict):
    nc = tc.nc
    xT = aps["xT"]          # [F, NTOK]
    l0r, l1r, l2r = aps["l0r"], aps["l1r"], aps["l2r"]  # [128, WFREE]
    l0i, l1i, l2i = aps["l0i"], aps["l1i"], aps["l2i"]
    phase = aps["phase"]    # [1, NBATCH]
    out = aps["out"]        # [NTOK, RSH]

    RSH, NK, NB = cfg.RSH, cfg.NK, cfg.NBATCH
    TPG = cfg.TOKG // 128   # psum tiles per token group
    fp16 = cfg.PREC == "fp16"
    DT = cfg.DT

    wpool = ctx.enter_context(tc.tile_pool(name="w", bufs=1))
    xpool = ctx.enter_context(tc.tile_pool(name="x", bufs=8))
    tpool = ctx.enter_context(tc.tile_pool(name="tmp", bufs=3))
    spool = ctx.enter_context(tc.tile_pool(name="stage", bufs=3))
    mpool = ctx.enter_context(tc.tile_pool(name="misc", bufs=1))
    pspool = ctx.enter_context(tc.tile_pool(name="ps", bufs=2, space="PSUM"))
    if not fp16:
        xrpool = ctx.enter_context(tc.tile_pool(name="xr", bufs=4))

    # --- cos(phase) on device: fold phase+pi/2 into [-pi, pi], then Sin LUT
    ph = mpool.tile([128, NB], F32)
    nc.sync.dma_start(out=ph[:], in_=phase[:1, :].to_broadcast([128, NB]))
    q = mpool.tile([128, NB], F32)
    nc.vector.tensor_scalar_add(q[:], ph[:], math.pi / 2)
    msk = mpool.tile([128, NB], F32)
    nc.vector.tensor_scalar(
        out=msk[:], in0=q[:], scalar1=math.pi, scalar2=2 * math.pi,
        op0=mybir.AluOpType.is_gt, op1=mybir.AluOpType.mult,
    )
    nc.vector.tensor_tensor(out=q[:], in0=q[:], in1=msk[:],
                            op=mybir.AluOpType.subtract)
    cos_t = mpool.tile([128, NB], F32)
    nc.scalar.activation(cos_t[:], q[:], mybir.ActivationFunctionType.Sin)

    if fp16:
        # --- assemble Wr and Wi slices in SBUF (fp16). Assembly copies ride
        # the scalar HWDGE ring so the sync ring is free for xt loads.
        WR = wpool.tile([128, cfg.WFREE], DT)
        WI = wpool.tile([128, cfg.WFREE], DT)
        gw = cfg.ASM_GRP * RSH
        if cfg.ASM_MODE == "dve":
            # plain full-rate DMAs; duplicate-layer summation on DVE
            lpool = ctx.enter_context(tc.tile_pool(name="lscr", bufs=6))
            dma_rr = 0
            for g in range(NK // cfg.ASM_GRP):
                sl = slice(g * gw, (g + 1) * gw)
                for W, ls in ((WR, (l0r, l1r, l2r)), (WI, (l0i, l1i, l2i))):
                    nc.scalar.dma_start(out=W[:, sl], in_=ls[0][:, sl])
                    for l_ap in ls[1:]:
                        scr = lpool.tile([128, gw], DT, name="scr")
                        eng = nc.sync if (dma_rr % 2 == 0) else nc.scalar
                        dma_rr += 1
                        eng.dma_start(out=scr[:], in_=l_ap[:, sl])
                        nc.vector.tensor_tensor(out=W[:, sl], in0=W[:, sl],
                                                in1=scr[:], op=ADD)
        else:
            for g in range(NK // cfg.ASM_GRP):
                sl = slice(g * gw, (g + 1) * gw)
                nc.scalar.dma_start(out=WR[:, sl], in_=l0r[:, sl])
                nc.gpsimd.dma_start(out=WR[:, sl], in_=l1r[:, sl],
                                    accum_op=ADD)
                nc.gpsimd.dma_start(out=WR[:, sl], in_=l2r[:, sl],
                                    accum_op=ADD)
                nc.scalar.dma_start(out=WI[:, sl], in_=l0i[:, sl])
                nc.gpsimd.dma_start(out=WI[:, sl], in_=l1i[:, sl],
                                    accum_op=ADD)
                nc.gpsimd.dma_start(out=WI[:, sl], in_=l2i[:, sl],
                                    accum_op=ADD)
        WB = wpool.tile([128, cfg.WFREE], DT)
    else:
        # f32r: WB doubles as Wr accumulator, combined incrementally.
        # DMA-written (unrounded) WB is only consumed by the DVE combine,
        # which rewrites it f32r-rounded before any matmul reads it.
        WB = wpool.tile([128, cfg.WFREE], F32R)
        WI = wpool.tile([128, cfg.WFREE], F32)
        gw = cfg.ASM_GRP * RSH
        for g in range(NK // cfg.ASM_GRP):
            sl = slice(g * gw, (g + 1) * gw)
            nc.scalar.dma_start(out=WB[:, sl], in_=l0r[:, sl].bitcast(F32R))
            nc.gpsimd.dma_start(out=WB[:, sl], in_=l1r[:, sl].bitcast(F32R),
                                accum_op=ADD)
            nc.gpsimd.dma_start(out=WB[:, sl], in_=l2r[:, sl].bitcast(F32R),
                                accum_op=ADD)
            nc.scalar.dma_start(out=WI[:, sl], in_=l0i[:, sl])
            nc.gpsimd.dma_start(out=WI[:, sl], in_=l1i[:, sl], accum_op=ADD)
            nc.gpsimd.dma_start(out=WI[:, sl], in_=l2i[:, sl], accum_op=ADD)

    # delta[b]: fp16 path uses absolute combine (WB = WR + cos_b*WI);
    # f32r path uses incremental (WB += (cos_b - cos_{b-1})*WI).
    dlt = mpool.tile([128, NB], F32)
    if fp16:
        nc.vector.tensor_copy(dlt[:], cos_t[:])
    else:
        nc.vector.tensor_copy(dlt[:, 0:1], cos_t[:, 0:1])
        if NB > 1:
            nc.vector.tensor_tensor(out=dlt[:, 1:NB], in0=cos_t[:, 1:NB],
                                    in1=cos_t[:, 0:NB - 1],
                                    op=mybir.AluOpType.subtract)

    # --- per batch: build W_b, then matmul all tokens of the batch
    ntg_per_b = cfg.NTG // NB
    for b in range(NB):
        for k in range(NK):
            sl = slice(k * RSH, (k + 1) * RSH)
            tmp = tpool.tile([128, RSH], DT)
            nc.vector.tensor_scalar(out=tmp[:], in0=WI[:, sl],
                                    scalar1=dlt[:, b:b + 1], scalar2=None,
                                    op0=mybir.AluOpType.mult)
            if fp16:
                nc.vector.tensor_tensor(out=WB[:, sl], in0=WR[:, sl],
                                        in1=tmp[:], op=ADD)
            else:
                nc.vector.tensor_tensor(out=WB[:, sl],
                                        in0=WB[:, sl].bitcast(F32),
                                        in1=tmp[:], op=ADD)
        for tg in range(ntg_per_b):
            gt = b * ntg_per_b + tg
            pts = [pspool.tile([128, RSH], F32, space="PSUM", tag=f"ps{t}",
                               name=f"ps{t}")
                   for t in range(TPG)]
            for k in range(NK):
                xt = xpool.tile([128, cfg.TOKG], DT)
                dma_eng = nc.sync if (k % 2 == 0) else nc.scalar
                row0 = (k * cfg.NTG + gt) * 128
                dma_eng.dma_start(out=xt[:], in_=xT[row0:row0 + 128, :])
                if fp16:
                    lhs_tile = xt
                else:
                    lhs_tile = xrpool.tile([128, cfg.TOKG], F32R, name="xtr")
                    nc.scalar.activation(lhs_tile[:], xt[:],
                                         mybir.ActivationFunctionType.Copy)
                for t in range(TPG):
                    nc.tensor.matmul(
                        out=pts[t][:],
                        lhsT=lhs_tile[:, t * 128:(t + 1) * 128],
                        rhs=WB[:, k * RSH:(k + 1) * RSH],
                        start=(k == 0), stop=(k == NK - 1),
                    )
            for t in range(TPG):
                stg = spool.tile([128, RSH], F32)
                nc.scalar.copy(out=stg[:], in_=pts[t][:])
                tok0 = gt * cfg.TOKG + t * 128
                nc.gpsimd.dma_start(out=out[tok0:tok0 + 128, :], in_=stg[:])


def build_nc(cfg: Cfg):
    nc = bacc.Bacc("TRN2", target_bir_lowering=False, debug=False,
                   num_devices=cfg.NCORES)
    aps = {
        # xT pre-tiled on host: row block (k*NTG + gt)*128 holds the
        # [128 feat, TOKG tok] tile for feature-chunk k, token-group gt.
        "xT": nc.dram_tensor("xT", [cfg.NK * cfg.NTG * 128, cfg.TOKG], cfg.DT,
                             kind="ExternalInput").ap(),
        "phase": nc.dram_tensor("phase", [1, cfg.NBATCH], F32,
                                kind="ExternalInput").ap(),
        "out": nc.dram_tensor("out", [cfg.NTOK, cfg.RSH], F32,
                              kind="ExternalOutput").ap(),
    }
    for name in ("l0r", "l1r", "l2r", "l0i", "l1i", "l2i"):
        aps[name] = nc.dram_tensor(name, [128, cfg.WFREE], cfg.DT,
                                   kind="ExternalInput").ap()
    with tile.TileContext(nc) as tc:
        with ExitStack() as ctx:
            build_body(ctx, tc, cfg, aps)
    nc.compile()
    return nc


def host_prep(cfg: Cfg, x, rows, cols, w_real, w_imag, phase_angles):
    """Pure-layout host prep: transpose x; sort COO edges by cell and place
    values into 3 per-generation dense layers in the on-chip W.T layout.
    Returns per-core input maps."""
    x = np.ascontiguousarray(np.asarray(x, dtype=np.float32)).reshape(
        cfg.NTOK, cfg.F)
    xT = x.T.astype(cfg.DT_NP)  # [F, NTOK]
    # pre-tile: row block (k*NTG + gt)*128 = [128 feat, TOKG tok] tile
    xT = np.ascontiguousarray(
        xT.reshape(cfg.NK, 128, cfg.NTG, cfg.TOKG).transpose(0, 2, 1, 3)
    ).reshape(cfg.NK * cfg.NTG * 128, cfg.TOKG)

    rows = np.asarray(rows).astype(np.int32, copy=False)
    cols = np.asarray(cols).astype(np.int32, copy=False)
    w_real = np.asarray(w_real, dtype=cfg.DT_NP)
    w_imag = np.asarray(w_imag, dtype=cfg.DT_NP)

    colbits = int(np.log2(cfg.F))
    lin = (rows.astype(np.int64) << colbits) | cols
    if cfg.RTOT * cfg.F <= 2**31:
        lin = lin.astype(np.int32)
    order = np.argsort(lin, kind="stable")
    sl = lin[order]
    wr_s = w_real[order]
    wi_s = w_imag[order]

    n = len(sl)
    starts = np.empty(n, dtype=bool)
    starts[0] = True
    starts[1:] = sl[1:] != sl[:-1]
    idx = np.arange(n, dtype=np.int64)
    gen = idx - np.maximum.accumulate(np.where(starts, idx, 0))

    r = (sl.astype(np.int64) >> colbits)
    c = (sl.astype(np.int64) & (cfg.F - 1))
    rsh_bits = int(np.log2(cfg.RSH))
    core = r >> rsh_bits
    p = c & 127
    off = ((c >> 7) << rsh_bits) + (r & (cfg.RSH - 1))

    shp = (cfg.NCORES, 128, cfg.WFREE)
    layers = {name: np.zeros(shp, dtype=cfg.DT_NP)
              for name in ("l0r", "l1r", "l2r", "l0i", "l1i", "l2i")}
    for g, (nr, ni) in enumerate((("l0r", "l0i"), ("l1r", "l1i"))):
        m = gen == g
        layers[nr][core[m], p[m], off[m]] = wr_s[m]
        layers[ni][core[m], p[m], off[m]] = wi_s[m]
    m = gen >= 2
    np.add.at(layers["l2r"], (core[m], p[m], off[m]), wr_s[m])
    np.add.at(layers["l2i"], (core[m], p[m], off[m]), wi_s[m])

    phase_in = np.asarray(phase_angles, dtype=np.float32).reshape(1, cfg.NBATCH)

    in_maps = []
    for cid in range(cfg.NCORES):
        m = {"xT": xT, "phase": phase_in}
        for name, arr in layers.items():
            m[name] = arr[cid]
        in_maps.append(m)
    return in_maps


_NC_CACHE = {}
LAST_RESULTS = None  # BassKernelResults of the most recent kernel() call


def kernel(x, rows, cols, w_real, w_imag, phase_angles, out_features=4096,
           **_ignored):
    from concourse.bass_utils import run_bass_kernel_spmd

    global LAST_RESULTS
    cfg = Cfg()
    assert int(out_features) == cfg.RTOT

    if "nc" not in _NC_CACHE:
        _NC_CACHE["nc"] = build_nc(cfg)
    nc = _NC_CACHE["nc"]

    in_maps = host_prep(cfg, x, rows, cols, w_real, w_imag, phase_angles)
    res = run_bass_kernel_spmd(nc, in_maps, core_ids=list(range(cfg.NCORES)))
    LAST_RESULTS = res
    out = np.concatenate([res.results[c]["out"] for c in range(cfg.NCORES)],
                         axis=1)
    return out.reshape(cfg.NTOK // 2048, 2048, cfg.RTOT)
